# revision 17
# baseline (speedup 1.0000x reference)
"""BERT self-attention (B=4, S=2048, H=768, 12 heads x 64) on 8 trn2 cores.

Sharding: core c = batch (c//2) x head-half (c%2, 6 heads each).
Each core computes Q/K/V projections for its 6 heads, attention, and a
partial output projection (its heads' slice of Wo). Host sums the two
partials per batch (plus a small hp2-sq3 tail partial, "out2") and adds bo.

All matmuls are bf16. HW notes that shaped this design (measured with
micro-benchmarks on the real silicon, see mm_bench*.py):
  - PE streams 1 output column/cycle at ~2.3 GHz regardless of dtype;
    fp8 DoubleRow/DoubleColumn/DoublePixel give NO speedup, and heavy
    fp8 trips the DSS power throttle to ~50% utilization (worse).
  - The two 64-contraction score matmuls of a head pair run CONCURRENTLY
    when placed at PE row-tiles (0,0)/(64,0) (~121ns each) -- the only
    matmul-level parallelism available.  Full-128-contraction matmuls
    are optimal back-to-back.
  - ACT exp costs ~1146ns per [128,1024] tile, no dtype/location levers.
  - A matmul's PSUM output may not exceed one bank (512 fp32 cols).

Per-core engine floors: PE ~245us busy, ACT ~220us; the attention loop
runs co-paced at ~1230ns/slot (slot = hp,sq,kt = one exp tile).

On-device layout (per core):
  xt   [768, 2048]  bf16  (DMA-transposed x)
  QT/KT per head-pair [128=2x64, 2048] bf16 (head-dim on partitions)
  scores^T [128 keys, 2x512 q] fp32 PSUM (two heads via PE row tiling)
  exp on ScalarE (scale=1/8, bias=mask column), out bf16
  attn@V -> comb [65, 512] PSUM; row 64 = softmax denominator
  combt per head pair [128, 2048] bf16 (head B staged via SBUF DMA),
    scaled by 1/denom
  out-proj: 3 x K=128 chunks (head pair merged) accumulated in SBUF
    out_acc fp32; last generation (hp2, sq3) instead uses two
    64-contraction phases straight off the normalize tiles into "out2"
    to keep the tail off the combt-DMA critical path.

The in-order PE is kept busy by (a) pipelining the score matmuls one
slot ahead, and (b) injecting independent projection matmuls into the
exp-wait bubble via per-hp deadline-ordered queues (V-chain for key
tile k must finish before slot k; kt-chain j before slot 4j-1).
"""

import numpy as np
import ml_dtypes

B, S, H = 4, 2048, 768
NH, HS = 12, 64
NHL = 6              # heads per core
NHP = 3              # head pairs per core
HCHUNKS = 6          # 768 / 128 contraction chunks
SKT = 16             # key tiles of 128
SQT = 4              # query tiles of 512
QW = 512             # query tile width
N_CORES = 8

SCORES_FP8 = False
ATTNV_FP8 = False
OUTPROJ_PAIR = True
DVE_EXP_KT = ()   # slots whose exp runs on DVE (Schraudolph bf16)
SCH_A = 128.0 / float(np.log(2.0))
SCH_B = 127.0 * 128.0 - 5.5

_COMPILED = None


def _build():
    import concourse.bass as bass
    import concourse.mybir as mybir
    import concourse.tile as tile
    from concourse import bacc

    fp32 = mybir.dt.float32
    bf16 = mybir.dt.bfloat16
    f8e4 = mybir.dt.float8e4
    AF = mybir.ActivationFunctionType
    DR = mybir.MatmulPerfMode.DoubleRow

    nc = bacc.Bacc("TRN2", target_bir_lowering=False, debug=False)

    xt_d = nc.dram_tensor("xt", [H, S], bf16, kind="ExternalInput").ap()
    wq_d = nc.dram_tensor("wq", [H, NHL * HS], bf16, kind="ExternalInput").ap()
    wk_d = nc.dram_tensor("wk", [H, NHL * HS], bf16, kind="ExternalInput").ap()
    wv_d = nc.dram_tensor("wv", [H, NHL * HS], bf16, kind="ExternalInput").ap()
    wo_d = nc.dram_tensor("wo", [NHL * HS, H], bf16, kind="ExternalInput").ap()
    bq_d = nc.dram_tensor("bq", [128, NHP], fp32, kind="ExternalInput").ap()
    bk_d = nc.dram_tensor("bk", [128, NHP], fp32, kind="ExternalInput").ap()
    bv_d = nc.dram_tensor("bv", [128, NHL * HS], fp32, kind="ExternalInput").ap()
    mask_d = nc.dram_tensor("mask", [128, SKT], fp32, kind="ExternalInput").ap()
    out_d = nc.dram_tensor("out", [S, H], fp32, kind="ExternalOutput").ap()
    out2_d = nc.dram_tensor("out2", [QW, H], fp32, kind="ExternalOutput").ap()

    with tile.TileContext(nc) as tc:
        with (
            tc.tile_pool(name="const", bufs=1) as const,
            tc.tile_pool(name="xt", bufs=1) as xtp,
            tc.tile_pool(name="vsb", bufs=1) as vsb,
            tc.tile_pool(name="qkt", bufs=2) as qkt,
            tc.tile_pool(name="combt", bufs=1) as combtp,
            tc.tile_pool(name="oacc", bufs=1) as oaccp,
            tc.tile_pool(name="attn", bufs=8 if not ATTNV_FP8 else 3) as attnp,
            tc.tile_pool(name="small", bufs=4) as smallp,
            tc.tile_pool(name="ps_sc", bufs=2, space="PSUM") as ps_sc,
            tc.tile_pool(name="ps_cb", bufs=2, space="PSUM") as ps_cb,
            tc.tile_pool(name="ps_pj", bufs=2, space="PSUM") as ps_pj,
        ):
            # ---- startup DMAs, alternating between two HWDGE queues,
            # in consumption order ----
            _dma_i = [0]
            _ld_engs = None

            def ld(dst, srcap):
                engs = _ld_engs or (nc.sync, nc.scalar, nc.gpsimd)
                engs[_dma_i[0] % len(engs)].dma_start(dst, srcap)
                _dma_i[0] += 1

            xt = [[None] * SQT for _ in range(HCHUNKS)]
            for piece in range(SQT):
                for c in range(HCHUNKS):
                    t = xtp.tile([128, QW], bf16, tag=f"xt{c}_{piece}",
                                 name=f"xt{c}_{piece}")
                    xt[c][piece] = t
            wv_sb, wq_sb, wk_sb = [], [], []
            for c in range(HCHUNKS):
                wv_sb.append(const.tile([128, NHL * HS], bf16, tag=f"wv{c}", name=f"wv{c}"))
                wq_sb.append(const.tile([128, NHL * HS], bf16, tag=f"wq{c}", name=f"wq{c}"))
                wk_sb.append(const.tile([128, NHL * HS], bf16, tag=f"wk{c}", name=f"wk{c}"))
            bq_sb = const.tile([128, NHP], fp32, tag="bq")
            bk_sb = const.tile([128, NHP], fp32, tag="bk")
            bv_sb = const.tile([128, NHL * HS], fp32, tag="bv")
            mask_sb = const.tile([128, SKT], fp32, tag="mask")
            bcols_sb = const.tile([128, SKT], fp32, tag="bcols")
            wo_sb = [const.tile([128, H], bf16, tag=f"wo{c}", name=f"wo{c}")
                     for c in range(NHP)]
            # head-B wo rows at base partition 0 for the tail's unmerged path
            wo_bt = const.tile([64, H], bf16, tag="wobt")

            for c in range(HCHUNKS):
                ld(xt[c][0][:], xt_d[c * 128:(c + 1) * 128, 0:QW])
                ld(wk_sb[c][:], wk_d[c * 128:(c + 1) * 128, :])
            ld(bk_sb[:], bk_d[:])
            for c in range(HCHUNKS):
                ld(wq_sb[c][:], wq_d[c * 128:(c + 1) * 128, :])
            ld(bq_sb[:], bq_d[:])
            ld(mask_sb[:], mask_d[:])
            for c in range(HCHUNKS):
                ld(wv_sb[c][:], wv_d[c * 128:(c + 1) * 128, :])
            ld(bv_sb[:], bv_d[:])
            for c in range(HCHUNKS):
                ld(xt[c][1][:], xt_d[c * 128:(c + 1) * 128, QW:2 * QW])
            for c in range(HCHUNKS):
                ld(xt[c][2][:], xt_d[c * 128:(c + 1) * 128, 2 * QW:3 * QW])
            for c in range(HCHUNKS):
                ld(xt[c][3][:], xt_d[c * 128:(c + 1) * 128, 3 * QW:4 * QW])
            for c in range(NHP):
                ld(wo_sb[c][:], wo_d[c * 128:(c + 1) * 128, :])
            ld(wo_bt[:], wo_d[2 * 128 + 64:3 * 128, :])

            # ---- V projection tiles ----
            VW = HS + 16     # fp8 head stride padded for 16B-aligned DR pairs
            CBW = VW if ATTNV_FP8 else HS + 1
            if ATTNV_FP8:
                # v8[p][k, h, s, d] for kt = 2p+s; col 64 of each head = 1.0
                v_sb = [vsb.tile([128, NHL, 2, VW], f8e4, tag=f"v{p}",
                                 name=f"v{p}")
                        for p in range(SKT // 2)]
            else:
                v_sb = [vsb.tile([128, NHL, HS + 1], bf16, tag=f"v{kt}",
                                 name=f"v{kt}")
                        for kt in range(SKT)]

            def v_units(kt):
                st8 = {}

                def unit(c, st8=st8):
                    if c == 0:
                        st8["ps"] = ps_pj.tile([128, 512], fp32, tag="pj",
                                               name="psv")
                    nc.tensor.matmul(
                        st8["ps"][:, :NHL * HS],
                        lhsT=xt[c][kt // 4][:, (kt % 4) * 128:(kt % 4 + 1) * 128],
                        rhs=wv_sb[c][:],
                        start=(c == 0),
                        stop=(c == HCHUNKS - 1),
                    )
                    if c == HCHUNKS - 1:
                        if ATTNV_FP8:
                            vt = v_sb[kt // 2]
                            s = kt % 2
                            dst = vt[:, :, s, 0:HS]
                            ones = vt[:, :, s, HS:HS + 1]
                            pad = vt[:, :, s, HS + 1:VW]
                        else:
                            vt = v_sb[kt]
                            dst = vt[:, :, 0:HS]
                            ones = vt[:, :, HS:HS + 1]
                            pad = None
                        nc.vector.tensor_add(
                            dst,
                            st8["ps"][:, :NHL * HS].rearrange(
                                "p (h d) -> p h d", h=NHL),
                            bv_sb[:].rearrange("p (h d) -> p h d", h=NHL),
                        )
                        nc.vector.memset(ones, 1.0)
                        if pad is not None:
                            nc.vector.memset(pad, 0.0)

                return [lambda c=c: unit(c) for c in range(HCHUNKS)]

            # DVE fast-exp bias columns: B_cols = A*mask + B
            nc.vector.tensor_scalar(
                bcols_sb[:], mask_sb[:], SCH_A, SCH_B,
                mybir.AluOpType.mult, mybir.AluOpType.add,
            )

            for kt in range(4):
                for u in v_units(kt):
                    u()

            # combt per head pair: [128, S] (head A rows 0:64, head B 64:128)
            combt = [combtp.tile([128, S], bf16, tag=f"ct{hp}", name=f"ct{hp}")
                     for hp in range(NHP)]
            # partial output accumulator [128, st, 768]
            out_acc = oaccp.tile([128, S // 128, H], fp32, tag="oacc")

            def emit_qkt(hp):
                """Q^T/K^T projection for head pair hp. Returns (qt, kt,
                units, chains). fp8 layout: kt [128, SKT, 2, 128],
                qt [128, SQT, 2, QW], DR subtile 1 all-zero."""
                if SCORES_FP8:
                    qt_t = qkt.tile([128, SQT, 2, QW], f8e4, tag="qt", name=f"qt{hp}")
                    kt_t = qkt.tile([128, SKT, 2, 128], f8e4, tag="kt", name=f"kt{hp}")
                else:
                    qt_t = qkt.tile([128, S], bf16, tag="qt", name=f"qt{hp}")
                    kt_t = qkt.tile([128, S], bf16, tag="kt", name=f"kt{hp}")
                units = []
                chains = {}
                zeroed = {}
                for kind, dst, w_sb, b_sb in (("kt", kt_t, wk_sb, bk_sb),
                                              ("qt", qt_t, wq_sb, bq_sb)):
                    for sq in range(SQT):
                        st8 = {}

                        def unit(c, kind=kind, dst=dst, w_sb=w_sb, b_sb=b_sb,
                                 sq=sq, st8=st8):
                            if c == 0:
                                if SCORES_FP8 and hp < 2 and not zeroed.get(kind):
                                    zeroed[kind] = True
                                    nc.gpsimd.memset(dst[:, :, 1, :], 0.0)
                                st8["ps"] = ps_pj.tile(
                                    [128, 512], fp32, tag="pj", name="psq")
                            nc.tensor.matmul(
                                st8["ps"][:],
                                lhsT=w_sb[c][:, hp * 128:(hp + 1) * 128],
                                rhs=xt[c][sq][:],
                                start=(c == 0),
                                stop=(c == HCHUNKS - 1),
                            )
                            if c == HCHUNKS - 1:
                                if SCORES_FP8:
                                    if kind == "qt":
                                        dslice = dst[:, sq, 0, :]
                                    else:
                                        dslice = dst[:, 4 * sq:4 * (sq + 1), 0, :]
                                else:
                                    dslice = dst[:, sq * QW:(sq + 1) * QW]
                                nc.vector.tensor_scalar_add(
                                    dslice, st8["ps"][:], b_sb[:, hp:hp + 1],
                                )

                        chain = [lambda c=c, u=unit: u(c)
                                 for c in range(HCHUNKS)]
                        chains[(kind, sq)] = chain
                        units.extend(chain)
                return qt_t, kt_t, units, chains

            def emit_outproj_unit(hp, st, half, phase, st8, stream_out=False):
                """Out-proj for (st, half) of head pair hp.
                OUTPROJ_PAIR: one 128-contraction matmul (phase 0 only,
                phase 1 = drain). Legacy: two 64-contraction matmuls."""
                if OUTPROJ_PAIR:
                    if phase == 0:
                        st8["ps"] = ps_pj.tile([128, 512], fp32, tag="pj", name="pso")
                        nc.tensor.matmul(
                            st8["ps"][:, 0:384],
                            lhsT=combt[hp][:, st * 128:(st + 1) * 128],
                            rhs=wo_sb[hp][:, half * 384:(half + 1) * 384],
                            start=True, stop=True,
                        )
                        return
                else:
                    if phase == 0:
                        st8["ps"] = ps_pj.tile([128, 512], fp32, tag="pj", name="pso")
                    ct_rows = combt[hp][0:64] if phase == 0 else combt[hp][64:128]
                    nc.tensor.matmul(
                        st8["ps"][:, 0:384],
                        lhsT=ct_rows[:, st * 128:(st + 1) * 128],
                        rhs=wo_sb[2 * hp + phase][:, half * 384:(half + 1) * 384],
                        start=(phase == 0), stop=(phase == 1),
                    )
                    if phase == 0:
                        return
                dst = out_acc[:, st, half * 384:(half + 1) * 384]
                if hp == 0:
                    nc.vector.tensor_copy(dst, st8["ps"][:, 0:384])
                else:
                    nc.vector.tensor_add(dst, dst, st8["ps"][:, 0:384])
                if stream_out:
                    nc.sync.dma_start(
                        out_d[st * 128:(st + 1) * 128, :], out_acc[:, st, :])

            def outproj_units(hp, sqs, stream_out=False, min_sq=None):
                """(min_sq, unit) out-proj work for the s-tiles inside query
                tiles `sqs` of head pair hp, gated one sq later (or at an
                explicit min_sq when queued into a later head pair)."""
                units = []
                for sq in sqs:
                    gate = sq + 2 if min_sq is None else min_sq
                    for st in range(4 * sq, 4 * (sq + 1)):
                        for half in range(2):
                            st8 = {}
                            for phase in range(2):
                                units.append((gate, lambda hp=hp, st=st,
                                              half=half, phase=phase, st8=st8,
                                              so=stream_out and phase == 1 and half == 1:
                                              emit_outproj_unit(hp, st, half, phase, st8, so)))
                return units

            qkts = [emit_qkt(0)]
            ch0 = qkts[0][3]
            for u in ch0[("kt", 0)] + ch0[("qt", 0)]:
                u()
            for u in ch0[("kt", 1)]:
                u()

            # per-hp injection queues: (min_sq, emit_fn).
            # V-chain for key-tile k must complete before slot k's attn@V,
            # and kt-chain j before slot 4j-1 (score lookahead), so the
            # queue is ordered by deadline.
            inject_q = {0: [], 1: [], 2: []}
            qkts.append(emit_qkt(1))
            inject_q[0] = (
                [(0, u) for u in v_units(4)]
                + [(0, u) for u in v_units(5)]
                + [(0, u) for u in ch0[("kt", 2)]]
                + [(0, u) for u in v_units(6)]
                + [(0, u) for u in v_units(7)]
                + [(0, u) for u in v_units(8)]
                + [(0, u) for u in ch0[("kt", 3)]]
                + [(0, u) for u in v_units(9)]
                + [(0, u) for u in v_units(10)]
                + [(0, u) for u in ch0[("qt", 1)]]
                + [(0, u) for kt in range(11, SKT) for u in v_units(kt)]
                + [(1, u) for u in ch0[("qt", 2)]]
                + [(2, u) for u in ch0[("qt", 3)]]
                + [(0, u) for u in qkts[1][2]]
                + outproj_units(0, range(SQT - 2)))

            slots = [(hp, sq, kt) for hp in range(NHP) for sq in range(SQT)
                     for kt in range(SKT)]

            def scores(hp, sq, kt):
                qt_t, kt_t = qkts[hp][0], qkts[hp][1]
                sc = ps_sc.tile([128, 1024], fp32, tag="sc", name="sc")
                if SCORES_FP8:
                    nc.tensor.matmul(
                        sc[:, 0:512],
                        lhsT=kt_t[0:64, kt, :, :],
                        rhs=qt_t[0:64, sq, :, :],
                        start=True, stop=True, perf_mode=DR,
                    )
                    nc.tensor.matmul(
                        sc[:, 512:1024],
                        lhsT=kt_t[64:128, kt, :, :],
                        rhs=qt_t[64:128, sq, :, :],
                        start=True, stop=True, perf_mode=DR,
                    )
                else:
                    nc.tensor.matmul(
                        sc[:, 0:512],
                        lhsT=kt_t[0:64, kt * 128:(kt + 1) * 128],
                        rhs=qt_t[0:64, sq * QW:(sq + 1) * QW],
                        start=True, stop=True,
                    )
                    nc.tensor.matmul(
                        sc[:, 512:1024],
                        lhsT=kt_t[64:128, kt * 128:(kt + 1) * 128],
                        rhs=qt_t[64:128, sq * QW:(sq + 1) * QW],
                        start=True, stop=True,
                    )
                return sc

            cb_cur = None
            at_cur = None
            tail_stage = [None]
            sc_cur = scores(*slots[0])
            for i, (hp, sq, kt) in enumerate(slots):
                if kt == 0:
                    if sq == 0 and hp > 0:
                        # drain any leftover injected work of the previous hp
                        for _, u in inject_q[hp - 1]:
                            u()
                        inject_q[hp - 1] = []
                    # build hp-level injection queues lazily at hp start
                    if sq == 0 and hp == 1:
                        qkts.append(emit_qkt(2))
                        inject_q[1] = (outproj_units(0, [SQT - 2, SQT - 1], min_sq=0)
                                       + [(0, u) for u in qkts[2][2]]
                                       + outproj_units(1, range(SQT - 2)))

                    if sq == 0 and hp == 2:
                        inject_q[2] = (outproj_units(1, [SQT - 2], min_sq=0)
                                       + outproj_units(1, [SQT - 1], min_sq=0,
                                                       stream_out=True)
                                       + outproj_units(2, range(SQT - 2),
                                                       stream_out=True)
                                       + outproj_units(2, [SQT - 2],
                                                       stream_out=True,
                                                       min_sq=SQT - 1))
                    cb_a = ps_cb.tile([CBW, 512], fp32, tag="cb", name="cba")
                    cb_b = ps_cb.tile([CBW, 512], fp32, tag="cb", name="cbb")
                    cb_cur = (cb_a, cb_b)
                # lookahead scores for the next slot
                sc_nxt = scores(*slots[i + 1]) if i + 1 < len(slots) else None
                if ATTNV_FP8:
                    if kt % 2 == 0:
                        at_cur = attnp.tile([128, 2, 2, 512], f8e4, tag="at")
                    at_dst = at_cur[:, :, kt % 2, :]
                else:
                    at_cur = attnp.tile([128, 1024], bf16, tag="at")
                    at_dst = at_cur[:]
                if kt in DVE_EXP_KT:
                    nc.vector.tensor_scalar(
                        at_cur.bitcast(mybir.dt.int16)[:], sc_cur[:],
                        SCH_A * 0.125, bcols_sb[:, kt:kt + 1],
                        mybir.AluOpType.mult, mybir.AluOpType.add,
                    )
                else:
                    nc.scalar.activation(
                        at_dst, sc_cur[:], AF.Exp,
                        bias=mask_sb[:, kt:kt + 1], scale=0.125,
                    )
                # fill the PE exp-wait bubble with independent work
                # (scan past gated units so a blocked head doesn't starve
                # eligible work behind it)
                q = inject_q[hp]
                popped = 0
                max_pop = 6 if (hp == 0 and sq == 0) else (3 if (hp == 1 and sq == 0) else 2)
                j = 0
                while j < len(q) and popped < max_pop:
                    if q[j][0] <= sq:
                        q.pop(j)[1]()
                        popped += 1
                    else:
                        j += 1
                cb_a, cb_b = cb_cur
                if ATTNV_FP8:
                    if kt % 2 == 1:
                        p = kt // 2
                        nc.tensor.matmul(
                            cb_a[:],
                            lhsT=v_sb[p][:, 2 * hp, :, :],
                            rhs=at_cur[:, 0, :, :],
                            start=(p == 0), stop=(p == SKT // 2 - 1),
                            perf_mode=DR,
                        )
                        nc.tensor.matmul(
                            cb_b[:],
                            lhsT=v_sb[p][:, 2 * hp + 1, :, :],
                            rhs=at_cur[:, 1, :, :],
                            start=(p == 0), stop=(p == SKT // 2 - 1),
                            perf_mode=DR,
                        )
                else:
                    nc.tensor.matmul(
                        cb_a[:],
                        lhsT=v_sb[kt][:, 2 * hp, :],
                        rhs=at_cur[:, 0:512],
                        start=(kt == 0), stop=(kt == SKT - 1),
                    )
                    nc.tensor.matmul(
                        cb_b[:],
                        lhsT=v_sb[kt][:, 2 * hp + 1, :],
                        rhs=at_cur[:, 512:1024],
                        start=(kt == 0), stop=(kt == SKT - 1),
                    )
                sc_cur = sc_nxt
                if kt == SKT - 1:
                    # normalize: comb rows 0..63 / denom(row 64).
                    # Two quick copies free both PSUM banks before the
                    # slow recip/broadcast chains run. Head A lands in
                    # combt rows 0:64; head B is staged and DMA'd into
                    # rows 64:128 (engines cannot shift partitions).
                    last_gen = (hp == NHP - 1 and sq == SQT - 1)
                    cbs_list = []
                    for j2, cb in enumerate((cb_a, cb_b)):
                        cbs = smallp.tile([65, 512], fp32, tag="cbs", name="cbs")
                        if last_gen and j2 == 1:
                            nc.scalar.copy(cbs[:], cb[0:65, :])
                        else:
                            nc.vector.tensor_copy(cbs[:], cb[0:65, :])
                        cbs_list.append(cbs)
                    for j2, cbs in enumerate(cbs_list):
                        rc0 = smallp.tile([1, 512], fp32, tag="rc0")
                        nc.sync.dma_start(rc0[:], cbs[64:65, :])
                        rc1 = smallp.tile([1, 512], fp32, tag="rc1")
                        # approx recip is partition-0 only on HW
                        nc.vector.reciprocal_approx_fast(rc1[:], rc0[:])
                        bc = smallp.tile([64, 512], fp32, tag="bc")
                        nc.gpsimd.partition_broadcast(bc[:], rc1[:])
                        if j2 == 0:
                            nc.vector.tensor_mul(
                                combt[hp][0:64, sq * QW:(sq + 1) * QW],
                                cbs[0:64, :], bc[:],
                            )
                        else:
                            stage = smallp.tile([64, 512], bf16, tag="stg",
                                                name="stg")
                            nc.vector.tensor_mul(stage[:], cbs[0:64, :], bc[:])
                            if last_gen:
                                tail_stage[0] = stage
                            else:
                                nc.sync.dma_start(
                                    combt[hp][64:128, sq * QW:(sq + 1) * QW],
                                    stage[:])

            # ---- tail: leftovers, then hp2-sq3 out-proj as 2-phase
            # 64-contraction chains (head A from combt rows 0:64, head B
            # from the kept stage tile) streamed to out2 (host adds). ----
            for hp in range(NHP):
                for _, u in inject_q[hp]:
                    u()
                inject_q[hp] = []
            stage = tail_stage[0]
            for st in range(4 * (SQT - 1), 4 * SQT):
                for half in range(2):
                    ps = ps_pj.tile([128, 512], fp32, tag="pj", name="pst")
                    nc.tensor.matmul(
                        ps[:, 0:384],
                        lhsT=combt[2][0:64, st * 128:(st + 1) * 128],
                        rhs=wo_sb[2][0:64, half * 384:(half + 1) * 384],
                        start=True, stop=False,
                    )
                    nc.tensor.matmul(
                        ps[:, 0:384],
                        lhsT=stage[:, (st - 12) * 128:(st - 11) * 128],
                        rhs=wo_bt[:, half * 384:(half + 1) * 384],
                        start=False, stop=True,
                    )
                    tsb = smallp.tile([128, 384], fp32, tag="tsb", name="tsb")
                    if half == 0:
                        nc.vector.tensor_copy(tsb[:], ps[:, 0:384])
                    else:
                        nc.scalar.copy(tsb[:], ps[:, 0:384])
                    eng = nc.sync if half == 0 else nc.scalar
                    eng.dma_start(
                        out2_d[(st - 12) * 128:(st - 11) * 128,
                               half * 384:(half + 1) * 384],
                        tsb[:])

    nc.compile()
    return nc


def _get_compiled():
    global _COMPILED
    if _COMPILED is None:
        _COMPILED = _build()
    return _COMPILED


def _prep_core_inputs(x, mask, Wq, bq, Wk, bk, Wv, bv, Wo, core):
    b, hg = core // 2, core % 2
    lo, hi = hg * NHL * HS, (hg + 1) * NHL * HS
    bf = ml_dtypes.bfloat16
    return {
        "xt": np.ascontiguousarray(x[b].T).astype(bf),
        "wq": np.ascontiguousarray(Wq[:, lo:hi]).astype(bf),
        "wk": np.ascontiguousarray(Wk[:, lo:hi]).astype(bf),
        "wv": np.ascontiguousarray(Wv[:, lo:hi]).astype(bf),
        "wo": np.ascontiguousarray(Wo[lo:hi, :]).astype(bf),
        "bq": np.ascontiguousarray(bq[lo:hi].reshape(NHP, 128).T).astype(np.float32),
        "bk": np.ascontiguousarray(bk[lo:hi].reshape(NHP, 128).T).astype(np.float32),
        "bv": np.tile(bv[lo:hi][None, :], (128, 1)).astype(np.float32),
        "mask": np.ascontiguousarray(
            mask[b, 0, 0].reshape(SKT, 128).T).astype(np.float32),
    }


def kernel(x, additive_attention_mask, Wq, bq, Wk, bk, Wv, bv, Wo, bo):
    from concourse import bass2jax

    x = np.asarray(x, dtype=np.float32)
    mask = np.asarray(additive_attention_mask, dtype=np.float32)
    args = [np.asarray(a, dtype=np.float32) for a in (Wq, bq, Wk, bk, Wv, bv, Wo)]
    Wq, bq, Wk, bk, Wv, bv, Wo = args
    bo = np.asarray(bo, dtype=np.float32)

    nc = _get_compiled()
    in_maps = [
        _prep_core_inputs(x, mask, Wq, bq, Wk, bk, Wv, bv, Wo, c)
        for c in range(N_CORES)
    ]
    results = bass2jax.run_bass_via_pjrt(nc, in_maps, n_cores=N_CORES)

    out = np.empty((B, S, H), dtype=np.float32)
    for b in range(B):
        out[b] = results[2 * b]["out"] + results[2 * b + 1]["out"] + bo
        out[b][3 * QW:] += results[2 * b]["out2"] + results[2 * b + 1]["out2"]
    return out


# revision 19
# speedup vs baseline: 1.0006x; 1.0006x over previous
"""BERT self-attention (B=4, S=2048, H=768, 12 heads x 64) on 8 trn2 cores.

Sharding: core c = batch (c//2) x head-half (c%2, 6 heads each).
Each core computes Q/K/V projections for its 6 heads, attention, and a
partial output projection (its heads' slice of Wo). Host sums the two
partials per batch (plus a small hp2-sq3 tail partial, "out2") and adds bo.

All matmuls are bf16. HW notes that shaped this design (measured with
micro-benchmarks on the real silicon, see mm_bench*.py):
  - PE streams 1 output column/cycle at ~2.3 GHz regardless of dtype;
    fp8 DoubleRow/DoubleColumn/DoublePixel give NO speedup, and heavy
    fp8 trips the DSS power throttle to ~50% utilization (worse).
  - The two 64-contraction score matmuls of a head pair run CONCURRENTLY
    when placed at PE row-tiles (0,0)/(64,0) (~121ns each) -- the only
    matmul-level parallelism available.  Full-128-contraction matmuls
    are optimal back-to-back.
  - ACT exp costs ~1146ns per [128,1024] tile, no dtype/location levers.
  - A matmul's PSUM output may not exceed one bank (512 fp32 cols).

Per-core engine floors: PE ~245us busy, ACT ~220us; the attention loop
runs co-paced at ~1230ns/slot (slot = hp,sq,kt = one exp tile).

On-device layout (per core):
  xt   [768, 2048]  bf16  (DMA-transposed x)
  QT/KT per head-pair [128=2x64, 2048] bf16 (head-dim on partitions)
  scores^T [128 keys, 2x512 q] fp32 PSUM (two heads via PE row tiling)
  exp on ScalarE (scale=1/8, bias=mask column), out bf16
  attn@V -> comb [65, 512] PSUM; row 64 = softmax denominator
  combt per head pair [128, 2048] bf16 (head B staged via SBUF DMA),
    scaled by 1/denom
  out-proj: 3 x K=128 chunks (head pair merged) accumulated in SBUF
    out_acc fp32; last generation (hp2, sq3) instead uses two
    64-contraction phases straight off the normalize tiles into "out2"
    to keep the tail off the combt-DMA critical path.

The in-order PE is kept busy by (a) pipelining the score matmuls one
slot ahead, and (b) injecting independent projection matmuls into the
exp-wait bubble via per-hp deadline-ordered queues (V-chain for key
tile k must finish before slot k; kt-chain j before slot 4j-1).
"""

import numpy as np
import ml_dtypes

B, S, H = 4, 2048, 768
NH, HS = 12, 64
NHL = 6              # heads per core
NHP = 3              # head pairs per core
HCHUNKS = 6          # 768 / 128 contraction chunks
SKT = 16             # key tiles of 128
SQT = 4              # query tiles of 512
QW = 512             # query tile width
N_CORES = 8

SCORES_FP8 = False
ATTNV_FP8 = False
OUTPROJ_PAIR = True
DVE_EXP_KT = ()   # slots whose exp runs on DVE (Schraudolph bf16)
SCH_A = 128.0 / float(np.log(2.0))
SCH_B = 127.0 * 128.0 - 5.5

_COMPILED = None


def _build():
    import concourse.bass as bass
    import concourse.mybir as mybir
    import concourse.tile as tile
    from concourse import bacc

    fp32 = mybir.dt.float32
    bf16 = mybir.dt.bfloat16
    f8e4 = mybir.dt.float8e4
    AF = mybir.ActivationFunctionType
    DR = mybir.MatmulPerfMode.DoubleRow

    nc = bacc.Bacc("TRN2", target_bir_lowering=False, debug=False)

    xt_d = nc.dram_tensor("xt", [H, S], bf16, kind="ExternalInput").ap()
    wq_d = nc.dram_tensor("wq", [H, NHL * HS], bf16, kind="ExternalInput").ap()
    wk_d = nc.dram_tensor("wk", [H, NHL * HS], bf16, kind="ExternalInput").ap()
    wv_d = nc.dram_tensor("wv", [H, NHL * HS], bf16, kind="ExternalInput").ap()
    wo_d = nc.dram_tensor("wo", [NHL * HS, H], bf16, kind="ExternalInput").ap()
    bq_d = nc.dram_tensor("bq", [128, NHP], fp32, kind="ExternalInput").ap()
    bk_d = nc.dram_tensor("bk", [128, NHP], fp32, kind="ExternalInput").ap()
    bv_d = nc.dram_tensor("bv", [128, NHL * HS], fp32, kind="ExternalInput").ap()
    mask_d = nc.dram_tensor("mask", [128, SKT], fp32, kind="ExternalInput").ap()
    out_d = nc.dram_tensor("out", [S, H], fp32, kind="ExternalOutput").ap()
    out2_d = nc.dram_tensor("out2", [QW, H], fp32, kind="ExternalOutput").ap()

    with tile.TileContext(nc) as tc:
        with (
            tc.tile_pool(name="const", bufs=1) as const,
            tc.tile_pool(name="xt", bufs=1) as xtp,
            tc.tile_pool(name="vsb", bufs=1) as vsb,
            tc.tile_pool(name="qkt", bufs=2) as qkt,
            tc.tile_pool(name="combt", bufs=1) as combtp,
            tc.tile_pool(name="oacc", bufs=1) as oaccp,
            tc.tile_pool(name="attn", bufs=8 if not ATTNV_FP8 else 3) as attnp,
            tc.tile_pool(name="small", bufs=4) as smallp,
            tc.tile_pool(name="ps_sc", bufs=2, space="PSUM") as ps_sc,
            tc.tile_pool(name="ps_cb", bufs=2, space="PSUM") as ps_cb,
            tc.tile_pool(name="ps_pj", bufs=2, space="PSUM") as ps_pj,
        ):
            # ---- startup DMAs, alternating between two HWDGE queues,
            # in consumption order ----
            _dma_i = [0]
            _ld_engs = None

            def ld(dst, srcap):
                engs = _ld_engs or (nc.sync, nc.scalar, nc.gpsimd)
                engs[_dma_i[0] % len(engs)].dma_start(dst, srcap)
                _dma_i[0] += 1

            xt = [[None] * SQT for _ in range(HCHUNKS)]
            for piece in range(SQT):
                for c in range(HCHUNKS):
                    t = xtp.tile([128, QW], bf16, tag=f"xt{c}_{piece}",
                                 name=f"xt{c}_{piece}")
                    xt[c][piece] = t
            wv_sb, wq_sb, wk_sb = [], [], []
            for c in range(HCHUNKS):
                wv_sb.append(const.tile([128, NHL * HS], bf16, tag=f"wv{c}", name=f"wv{c}"))
                wq_sb.append(const.tile([128, NHL * HS], bf16, tag=f"wq{c}", name=f"wq{c}"))
                wk_sb.append(const.tile([128, NHL * HS], bf16, tag=f"wk{c}", name=f"wk{c}"))
            bq_sb = const.tile([128, NHP], fp32, tag="bq")
            bk_sb = const.tile([128, NHP], fp32, tag="bk")
            bv_sb = const.tile([128, NHL * HS], fp32, tag="bv")
            mask_sb = const.tile([128, SKT], fp32, tag="mask")
            bcols_sb = const.tile([128, SKT], fp32, tag="bcols")
            wo_sb = [const.tile([128, H], bf16, tag=f"wo{c}", name=f"wo{c}")
                     for c in range(NHP)]
            # head-B wo rows at base partition 0 for the tail's unmerged path
            wo_bt = const.tile([64, H], bf16, tag="wobt")

            for c in range(HCHUNKS):
                ld(xt[c][0][:], xt_d[c * 128:(c + 1) * 128, 0:QW])
                ld(wk_sb[c][:], wk_d[c * 128:(c + 1) * 128, :])
            ld(bk_sb[:], bk_d[:])
            for c in range(HCHUNKS):
                ld(wq_sb[c][:], wq_d[c * 128:(c + 1) * 128, :])
            ld(bq_sb[:], bq_d[:])
            ld(mask_sb[:], mask_d[:])
            for c in range(HCHUNKS):
                ld(wv_sb[c][:], wv_d[c * 128:(c + 1) * 128, :])
            ld(bv_sb[:], bv_d[:])
            for c in range(HCHUNKS):
                ld(xt[c][1][:], xt_d[c * 128:(c + 1) * 128, QW:2 * QW])
            for c in range(HCHUNKS):
                ld(xt[c][2][:], xt_d[c * 128:(c + 1) * 128, 2 * QW:3 * QW])
            for c in range(HCHUNKS):
                ld(xt[c][3][:], xt_d[c * 128:(c + 1) * 128, 3 * QW:4 * QW])
            for c in range(NHP):
                ld(wo_sb[c][:], wo_d[c * 128:(c + 1) * 128, :])
            ld(wo_bt[:], wo_d[2 * 128 + 64:3 * 128, :])

            # ---- V projection tiles ----
            VW = HS + 16     # fp8 head stride padded for 16B-aligned DR pairs
            CBW = VW if ATTNV_FP8 else HS + 1
            if ATTNV_FP8:
                # v8[p][k, h, s, d] for kt = 2p+s; col 64 of each head = 1.0
                v_sb = [vsb.tile([128, NHL, 2, VW], f8e4, tag=f"v{p}",
                                 name=f"v{p}")
                        for p in range(SKT // 2)]
            else:
                v_sb = [vsb.tile([128, NHL, HS + 1], bf16, tag=f"v{kt}",
                                 name=f"v{kt}")
                        for kt in range(SKT)]

            def v_units(kt):
                st8 = {}

                def unit(c, st8=st8):
                    if c == 0:
                        st8["ps"] = ps_pj.tile([128, 512], fp32, tag="pj",
                                               name="psv")
                    nc.tensor.matmul(
                        st8["ps"][:, :NHL * HS],
                        lhsT=xt[c][kt // 4][:, (kt % 4) * 128:(kt % 4 + 1) * 128],
                        rhs=wv_sb[c][:],
                        start=(c == 0),
                        stop=(c == HCHUNKS - 1),
                    )
                    if c == HCHUNKS - 1:
                        if ATTNV_FP8:
                            vt = v_sb[kt // 2]
                            s = kt % 2
                            dst = vt[:, :, s, 0:HS]
                            ones = vt[:, :, s, HS:HS + 1]
                            pad = vt[:, :, s, HS + 1:VW]
                        else:
                            vt = v_sb[kt]
                            dst = vt[:, :, 0:HS]
                            ones = vt[:, :, HS:HS + 1]
                            pad = None
                        nc.vector.tensor_add(
                            dst,
                            st8["ps"][:, :NHL * HS].rearrange(
                                "p (h d) -> p h d", h=NHL),
                            bv_sb[:].rearrange("p (h d) -> p h d", h=NHL),
                        )
                        nc.vector.memset(ones, 1.0)
                        if pad is not None:
                            nc.vector.memset(pad, 0.0)

                return [lambda c=c: unit(c) for c in range(HCHUNKS)]

            # DVE fast-exp bias columns: B_cols = A*mask + B
            nc.vector.tensor_scalar(
                bcols_sb[:], mask_sb[:], SCH_A, SCH_B,
                mybir.AluOpType.mult, mybir.AluOpType.add,
            )

            for kt in range(4):
                for u in v_units(kt):
                    u()

            # combt per head pair: [128, S] (head A rows 0:64, head B 64:128)
            combt = [combtp.tile([128, S], bf16, tag=f"ct{hp}", name=f"ct{hp}")
                     for hp in range(NHP)]
            # partial output accumulator [128, st, 768]
            out_acc = oaccp.tile([128, S // 128, H], fp32, tag="oacc")

            def emit_qkt(hp):
                """Q^T/K^T projection for head pair hp. Returns (qt, kt,
                units, chains). fp8 layout: kt [128, SKT, 2, 128],
                qt [128, SQT, 2, QW], DR subtile 1 all-zero."""
                if SCORES_FP8:
                    qt_t = qkt.tile([128, SQT, 2, QW], f8e4, tag="qt", name=f"qt{hp}")
                    kt_t = qkt.tile([128, SKT, 2, 128], f8e4, tag="kt", name=f"kt{hp}")
                else:
                    qt_t = qkt.tile([128, S], bf16, tag="qt", name=f"qt{hp}")
                    kt_t = qkt.tile([128, S], bf16, tag="kt", name=f"kt{hp}")
                units = []
                chains = {}
                zeroed = {}
                for kind, dst, w_sb, b_sb in (("kt", kt_t, wk_sb, bk_sb),
                                              ("qt", qt_t, wq_sb, bq_sb)):
                    for sq in range(SQT):
                        st8 = {}

                        def unit(c, kind=kind, dst=dst, w_sb=w_sb, b_sb=b_sb,
                                 sq=sq, st8=st8):
                            if c == 0:
                                if SCORES_FP8 and hp < 2 and not zeroed.get(kind):
                                    zeroed[kind] = True
                                    nc.gpsimd.memset(dst[:, :, 1, :], 0.0)
                                st8["ps"] = ps_pj.tile(
                                    [128, 512], fp32, tag="pj", name="psq")
                            nc.tensor.matmul(
                                st8["ps"][:],
                                lhsT=w_sb[c][:, hp * 128:(hp + 1) * 128],
                                rhs=xt[c][sq][:],
                                start=(c == 0),
                                stop=(c == HCHUNKS - 1),
                            )
                            if c == HCHUNKS - 1:
                                if SCORES_FP8:
                                    if kind == "qt":
                                        dslice = dst[:, sq, 0, :]
                                    else:
                                        dslice = dst[:, 4 * sq:4 * (sq + 1), 0, :]
                                else:
                                    dslice = dst[:, sq * QW:(sq + 1) * QW]
                                nc.vector.tensor_scalar_add(
                                    dslice, st8["ps"][:], b_sb[:, hp:hp + 1],
                                )

                        chain = [lambda c=c, u=unit: u(c)
                                 for c in range(HCHUNKS)]
                        chains[(kind, sq)] = chain
                        units.extend(chain)
                return qt_t, kt_t, units, chains

            def emit_outproj_unit(hp, st, half, phase, st8, stream_out=False):
                """Out-proj for (st, half) of head pair hp.
                OUTPROJ_PAIR: one 128-contraction matmul (phase 0 only,
                phase 1 = drain). Legacy: two 64-contraction matmuls."""
                if OUTPROJ_PAIR:
                    if phase == 0:
                        st8["ps"] = ps_pj.tile([128, 512], fp32, tag="pj", name="pso")
                        nc.tensor.matmul(
                            st8["ps"][:, 0:384],
                            lhsT=combt[hp][:, st * 128:(st + 1) * 128],
                            rhs=wo_sb[hp][:, half * 384:(half + 1) * 384],
                            start=True, stop=True,
                        )
                        return
                else:
                    if phase == 0:
                        st8["ps"] = ps_pj.tile([128, 512], fp32, tag="pj", name="pso")
                    ct_rows = combt[hp][0:64] if phase == 0 else combt[hp][64:128]
                    nc.tensor.matmul(
                        st8["ps"][:, 0:384],
                        lhsT=ct_rows[:, st * 128:(st + 1) * 128],
                        rhs=wo_sb[2 * hp + phase][:, half * 384:(half + 1) * 384],
                        start=(phase == 0), stop=(phase == 1),
                    )
                    if phase == 0:
                        return
                dst = out_acc[:, st, half * 384:(half + 1) * 384]
                if hp == 0:
                    nc.vector.tensor_copy(dst, st8["ps"][:, 0:384])
                else:
                    nc.vector.tensor_add(dst, dst, st8["ps"][:, 0:384])
                if stream_out:
                    nc.sync.dma_start(
                        out_d[st * 128:(st + 1) * 128, :], out_acc[:, st, :])

            def outproj_units(hp, sqs, stream_out=False, min_sq=None):
                """(min_sq, unit) out-proj work for the s-tiles inside query
                tiles `sqs` of head pair hp, gated one sq later (or at an
                explicit min_sq when queued into a later head pair)."""
                units = []
                for sq in sqs:
                    gate = sq + 2 if min_sq is None else min_sq
                    for st in range(4 * sq, 4 * (sq + 1)):
                        for half in range(2):
                            st8 = {}
                            for phase in range(2):
                                units.append((gate, lambda hp=hp, st=st,
                                              half=half, phase=phase, st8=st8,
                                              so=stream_out and phase == 1 and half == 1:
                                              emit_outproj_unit(hp, st, half, phase, st8, so)))
                return units

            qkts = [emit_qkt(0)]
            ch0 = qkts[0][3]
            for u in ch0[("kt", 0)] + ch0[("qt", 0)]:
                u()
            for u in ch0[("kt", 1)]:
                u()

            # per-hp injection queues: (min_sq, emit_fn).
            # V-chain for key-tile k must complete before slot k's attn@V,
            # and kt-chain j before slot 4j-1 (score lookahead), so the
            # queue is ordered by deadline.
            inject_q = {0: [], 1: [], 2: []}
            qkts.append(emit_qkt(1))
            inject_q[0] = (
                [(0, u) for u in v_units(4)]
                + [(0, u) for u in v_units(5)]
                + [(0, u) for u in ch0[("kt", 2)]]
                + [(0, u) for u in v_units(6)]
                + [(0, u) for u in v_units(7)]
                + [(0, u) for u in v_units(8)]
                + [(0, u) for u in ch0[("kt", 3)]]
                + [(0, u) for u in v_units(9)]
                + [(0, u) for u in v_units(10)]
                + [(0, u) for u in ch0[("qt", 1)]]
                + [(0, u) for kt in range(11, SKT) for u in v_units(kt)]
                + [(1, u) for u in ch0[("qt", 2)]]
                + [(2, u) for u in ch0[("qt", 3)]]
                + [(0, u) for u in qkts[1][2]]
                + outproj_units(0, range(SQT - 2)))

            slots = [(hp, sq, kt) for hp in range(NHP) for sq in range(SQT)
                     for kt in range(SKT)]

            def scores(hp, sq, kt):
                qt_t, kt_t = qkts[hp][0], qkts[hp][1]
                sc = ps_sc.tile([128, 1024], fp32, tag="sc", name="sc")
                if SCORES_FP8:
                    nc.tensor.matmul(
                        sc[:, 0:512],
                        lhsT=kt_t[0:64, kt, :, :],
                        rhs=qt_t[0:64, sq, :, :],
                        start=True, stop=True, perf_mode=DR,
                    )
                    nc.tensor.matmul(
                        sc[:, 512:1024],
                        lhsT=kt_t[64:128, kt, :, :],
                        rhs=qt_t[64:128, sq, :, :],
                        start=True, stop=True, perf_mode=DR,
                    )
                else:
                    nc.tensor.matmul(
                        sc[:, 0:512],
                        lhsT=kt_t[0:64, kt * 128:(kt + 1) * 128],
                        rhs=qt_t[0:64, sq * QW:(sq + 1) * QW],
                        start=True, stop=True,
                    )
                    nc.tensor.matmul(
                        sc[:, 512:1024],
                        lhsT=kt_t[64:128, kt * 128:(kt + 1) * 128],
                        rhs=qt_t[64:128, sq * QW:(sq + 1) * QW],
                        start=True, stop=True,
                    )
                return sc

            cb_cur = None
            at_cur = None
            tail_stage = [None]
            sc_cur = scores(*slots[0])
            for i, (hp, sq, kt) in enumerate(slots):
                if kt == 0:
                    if sq == 0 and hp > 0:
                        # drain any leftover injected work of the previous hp
                        for _, u in inject_q[hp - 1]:
                            u()
                        inject_q[hp - 1] = []
                    # build hp-level injection queues lazily at hp start
                    if sq == 0 and hp == 1:
                        qkts.append(emit_qkt(2))
                        inject_q[1] = (outproj_units(0, [SQT - 2, SQT - 1], min_sq=0)
                                       + [(0, u) for u in qkts[2][2]]
                                       + outproj_units(1, range(SQT - 2)))

                    if sq == 0 and hp == 2:
                        inject_q[2] = (outproj_units(1, [SQT - 2], min_sq=0)
                                       + outproj_units(1, [SQT - 1], min_sq=0,
                                                       stream_out=True)
                                       + outproj_units(2, range(SQT - 2),
                                                       stream_out=True)
                                       + outproj_units(2, [SQT - 2],
                                                       stream_out=True,
                                                       min_sq=SQT - 1))
                    cb_a = ps_cb.tile([CBW, 512], fp32, tag="cb", name="cba")
                    cb_b = ps_cb.tile([CBW, 512], fp32, tag="cb", name="cbb")
                    cb_cur = (cb_a, cb_b)
                # lookahead scores for the next slot
                sc_nxt = scores(*slots[i + 1]) if i + 1 < len(slots) else None
                if ATTNV_FP8:
                    if kt % 2 == 0:
                        at_cur = attnp.tile([128, 2, 2, 512], f8e4, tag="at")
                    at_dst = at_cur[:, :, kt % 2, :]
                else:
                    at_cur = attnp.tile([128, 1024], bf16, tag="at")
                    at_dst = at_cur[:]
                if kt in DVE_EXP_KT:
                    nc.vector.tensor_scalar(
                        at_cur.bitcast(mybir.dt.int16)[:], sc_cur[:],
                        SCH_A * 0.125, bcols_sb[:, kt:kt + 1],
                        mybir.AluOpType.mult, mybir.AluOpType.add,
                    )
                else:
                    nc.scalar.activation(
                        at_dst, sc_cur[:], AF.Exp,
                        bias=mask_sb[:, kt:kt + 1], scale=0.125,
                    )
                # fill the PE exp-wait bubble with independent work
                # (scan past gated units so a blocked head doesn't starve
                # eligible work behind it)
                q = inject_q[hp]
                popped = 0
                max_pop = 6 if (hp == 0 and sq == 0) else (3 if (hp == 1 and sq == 0) else 2)
                j = 0
                while j < len(q) and popped < max_pop:
                    if q[j][0] <= sq:
                        q.pop(j)[1]()
                        popped += 1
                    else:
                        j += 1
                cb_a, cb_b = cb_cur
                if ATTNV_FP8:
                    if kt % 2 == 1:
                        p = kt // 2
                        nc.tensor.matmul(
                            cb_a[:],
                            lhsT=v_sb[p][:, 2 * hp, :, :],
                            rhs=at_cur[:, 0, :, :],
                            start=(p == 0), stop=(p == SKT // 2 - 1),
                            perf_mode=DR,
                        )
                        nc.tensor.matmul(
                            cb_b[:],
                            lhsT=v_sb[p][:, 2 * hp + 1, :, :],
                            rhs=at_cur[:, 1, :, :],
                            start=(p == 0), stop=(p == SKT // 2 - 1),
                            perf_mode=DR,
                        )
                else:
                    nc.tensor.matmul(
                        cb_a[:],
                        lhsT=v_sb[kt][:, 2 * hp, :],
                        rhs=at_cur[:, 0:512],
                        start=(kt == 0), stop=(kt == SKT - 1),
                    )
                    nc.tensor.matmul(
                        cb_b[:],
                        lhsT=v_sb[kt][:, 2 * hp + 1, :],
                        rhs=at_cur[:, 512:1024],
                        start=(kt == 0), stop=(kt == SKT - 1),
                    )
                sc_cur = sc_nxt
                if kt == SKT - 1:
                    # normalize: comb rows 0..63 / denom(row 64).
                    # Two quick copies free both PSUM banks before the
                    # slow recip/broadcast chains run. Head A lands in
                    # combt rows 0:64; head B is staged and DMA'd into
                    # rows 64:128 (engines cannot shift partitions).
                    last_gen = (hp == NHP - 1 and sq == SQT - 1)
                    cbs_list = []
                    for j2, cb in enumerate((cb_a, cb_b)):
                        cbs = smallp.tile([65, 512], fp32, tag="cbs", name="cbs")
                        if last_gen and j2 == 1:
                            nc.scalar.copy(cbs[:], cb[0:65, :])
                        else:
                            nc.vector.tensor_copy(cbs[:], cb[0:65, :])
                        cbs_list.append(cbs)
                    for j2, cbs in enumerate(cbs_list):
                        rc0 = smallp.tile([1, 512], fp32, tag="rc0")
                        nc.sync.dma_start(rc0[:], cbs[64:65, :])
                        rc1 = smallp.tile([1, 512], fp32, tag="rc1")
                        # approx recip is partition-0 only on HW
                        nc.vector.reciprocal_approx_fast(rc1[:], rc0[:])
                        bc = smallp.tile([64, 512], fp32, tag="bc")
                        nc.gpsimd.partition_broadcast(bc[:], rc1[:])
                        if j2 == 0:
                            nc.vector.tensor_mul(
                                combt[hp][0:64, sq * QW:(sq + 1) * QW],
                                cbs[0:64, :], bc[:],
                            )
                        else:
                            stage = smallp.tile([64, 512], bf16, tag="stg",
                                                name="stg")
                            nc.vector.tensor_mul(stage[:], cbs[0:64, :], bc[:])
                            if last_gen:
                                tail_stage[0] = stage
                            else:
                                nc.sync.dma_start(
                                    combt[hp][64:128, sq * QW:(sq + 1) * QW],
                                    stage[:])

            # ---- tail: leftovers, then hp2-sq3 out-proj as 2-phase
            # 64-contraction chains (head A from combt rows 0:64, head B
            # from the kept stage tile) streamed to out2 (host adds). ----
            for hp in range(NHP):
                for _, u in inject_q[hp]:
                    u()
                inject_q[hp] = []
            stage = tail_stage[0]
            for st in range(4 * (SQT - 1), 4 * SQT):
                for half in range(2):
                    ps = ps_pj.tile([128, 512], fp32, tag="pj", name="pst")
                    nc.tensor.matmul(
                        ps[:, 0:384],
                        lhsT=combt[2][0:64, st * 128:(st + 1) * 128],
                        rhs=wo_sb[2][0:64, half * 384:(half + 1) * 384],
                        start=True, stop=False,
                    )
                    nc.tensor.matmul(
                        ps[:, 0:384],
                        lhsT=stage[:, (st - 12) * 128:(st - 11) * 128],
                        rhs=wo_bt[:, half * 384:(half + 1) * 384],
                        start=False, stop=True,
                    )
                    tsb = smallp.tile([128, 384], fp32, tag="tsb", name="tsb")
                    if half == 0:
                        nc.vector.tensor_copy(tsb[:], ps[:, 0:384])
                    else:
                        nc.scalar.copy(tsb[:], ps[:, 0:384])
                    eng = nc.sync if half == 0 else nc.scalar
                    eng.dma_start(
                        out2_d[(st - 12) * 128:(st - 11) * 128,
                               half * 384:(half + 1) * 384],
                        tsb[:])

    nc.compile()
    return nc


def _get_compiled():
    global _COMPILED
    if _COMPILED is None:
        _COMPILED = _build()
    return _COMPILED


def _prep_core_inputs(x, mask, Wq, bq, Wk, bk, Wv, bv, Wo, core):
    b, hg = core // 2, core % 2
    lo, hi = hg * NHL * HS, (hg + 1) * NHL * HS
    bf = ml_dtypes.bfloat16
    return {
        "xt": np.ascontiguousarray(x[b].T).astype(bf),
        "wq": np.ascontiguousarray(Wq[:, lo:hi]).astype(bf),
        "wk": np.ascontiguousarray(Wk[:, lo:hi]).astype(bf),
        "wv": np.ascontiguousarray(Wv[:, lo:hi]).astype(bf),
        "wo": np.ascontiguousarray(Wo[lo:hi, :]).astype(bf),
        "bq": np.ascontiguousarray(bq[lo:hi].reshape(NHP, 128).T).astype(np.float32),
        "bk": np.ascontiguousarray(bk[lo:hi].reshape(NHP, 128).T).astype(np.float32),
        "bv": np.tile(bv[lo:hi][None, :], (128, 1)).astype(np.float32),
        "mask": np.ascontiguousarray(
            mask[b, 0, 0].reshape(SKT, 128).T).astype(np.float32),
    }


def kernel(x, additive_attention_mask, Wq, bq, Wk, bk, Wv, bv, Wo, bo):
    from concourse import bass2jax

    x = np.asarray(x, dtype=np.float32)
    mask = np.asarray(additive_attention_mask, dtype=np.float32)
    args = [np.asarray(a, dtype=np.float32) for a in (Wq, bq, Wk, bk, Wv, bv, Wo)]
    Wq, bq, Wk, bk, Wv, bv, Wo = args
    bo = np.asarray(bo, dtype=np.float32)

    nc = _get_compiled()
    in_maps = [
        _prep_core_inputs(x, mask, Wq, bq, Wk, bk, Wv, bv, Wo, c)
        for c in range(N_CORES)
    ]
    results = bass2jax.run_bass_via_pjrt(nc, in_maps, n_cores=N_CORES)

    out = np.empty((B, S, H), dtype=np.float32)
    for b in range(B):
        out[b] = results[2 * b]["out"] + results[2 * b + 1]["out"] + bo
        out[b][3 * QW:] += results[2 * b]["out2"] + results[2 * b + 1]["out2"]
    return out
